# revision 26
# baseline (speedup 1.0000x reference)
"""GNN policy kernel for Trainium2 (Bass/Tile), 8-core data-parallel over batch.

Per core (one batch element, N=2048 nodes, feature dims on SBUF partitions):
  h1T = relu(W1.T @ xT);  h2T = relu(W2.T @ h1T)            [64, 2048]
  key[m, n] = h_m . h_n - 0.5*||h_n||^2   (= -dist/2 + const(m); same ordering)
      via K=65 matmul: lhsT = [h2T_mchunk; ones], rhs = [h2T; -0.5*sq]
  top-8 per row with DVE max/max_index -> knn idx (rank 0 = self, keep 1..3)
  gather neighbor h rows from DRAM via indirect DMA, PE-transpose,
  msg MLP relu(. @ Wm + bm), summed over k (mean folded into Wu1 scaling),
  u = relu(hc @ Wu1 + bu1); u = relu(u @ Wu2 + bu2)
  out3 = u @ [Wmean | Wv] + [bmean | bv]  -> [2048, 3] rows to DRAM.
Host splits mean/v and computes exp(log_std).
"""

import sys

if "/opt/trn_rl_repo" not in sys.path:
    sys.path.insert(0, "/opt/trn_rl_repo")

from contextlib import ExitStack

import numpy as np

import concourse.bacc as bacc
import concourse.bass as bass
import concourse.mybir as mybir
import concourse.tile as tile
from concourse import library_config
from concourse.masks import make_identity

B = 8
N = 2048
D0 = 4
D = 64
DU = 128
KNN = 3
PC = 128           # nodes per topk chunk (partition dim)
NCH = N // PC      # 16
NQ = 512           # matmul moving free size (one PSUM bank)
QCH = N // NQ      # 4

F32 = mybir.dt.float32
U32 = mybir.dt.uint32
U16 = mybir.dt.uint16
I16 = mybir.dt.int16
AF = mybir.ActivationFunctionType


def build_program(dbg=False):
    nc = bacc.Bacc("TRN2", target_bir_lowering=False, debug=False, num_devices=B)

    x_d = nc.dram_tensor("x", [N, D0], F32, kind="ExternalInput")
    W1_d = nc.dram_tensor("W1", [D0, D], F32, kind="ExternalInput")
    b1_d = nc.dram_tensor("b1", [D], F32, kind="ExternalInput")
    W2_d = nc.dram_tensor("W2", [D, D], F32, kind="ExternalInput")
    b2_d = nc.dram_tensor("b2", [D], F32, kind="ExternalInput")
    Wm_d = nc.dram_tensor("Wm", [D, D], F32, kind="ExternalInput")
    bm_d = nc.dram_tensor("bm", [D], F32, kind="ExternalInput")
    Wu1_d = nc.dram_tensor("Wu1", [DU, DU], F32, kind="ExternalInput")
    bu1_d = nc.dram_tensor("bu1", [DU], F32, kind="ExternalInput")
    Wu2_d = nc.dram_tensor("Wu2", [DU, DU], F32, kind="ExternalInput")
    bu2_d = nc.dram_tensor("bu2", [DU], F32, kind="ExternalInput")
    Wo_d = nc.dram_tensor("Wo", [DU, 3], F32, kind="ExternalInput")
    bo_d = nc.dram_tensor("bo", [3], F32, kind="ExternalInput")
    out3_d = nc.dram_tensor("out3", [N, 3], F32, kind="ExternalOutput")
    if dbg:
        dbg_h = nc.dram_tensor("dbg_h", [PC, NCH * D], F32, kind="ExternalOutput")
        dbg_key = nc.dram_tensor("dbg_key", [PC, N], F32, kind="ExternalOutput")
        dbg_vals = nc.dram_tensor("dbg_vals", [PC, NCH, 8], F32, kind="ExternalOutput")
        dbg_idx = nc.dram_tensor("dbg_idx", [PC, 8, NCH], U32, kind="ExternalOutput")
        dbg_neigh = nc.dram_tensor("dbg_neigh", [PC, NCH, D], F32, kind="ExternalOutput")
        dbg_msum = nc.dram_tensor("dbg_msum", [D, N], F32, kind="ExternalOutput")

    with tile.TileContext(nc) as tc, ExitStack() as ctx:
        cpool = ctx.enter_context(tc.tile_pool(name="consts", bufs=1))
        pers = ctx.enter_context(tc.tile_pool(name="persist", bufs=1))
        s64p = ctx.enter_context(tc.tile_pool(name="s64", bufs=2))
        keyp = ctx.enter_context(tc.tile_pool(name="key", bufs=2))
        v8p = ctx.enter_context(tc.tile_pool(name="v8", bufs=2))
        neighp = ctx.enter_context(tc.tile_pool(name="neigh", bufs=2))
        ntp = ctx.enter_context(tc.tile_pool(name="nT", bufs=2))
        msgp = ctx.enter_context(tc.tile_pool(name="msg", bufs=2))
        up = ctx.enter_context(tc.tile_pool(name="u", bufs=2))
        dramp = ctx.enter_context(tc.tile_pool(name="dram", bufs=1, space="DRAM"))
        psU = ctx.enter_context(tc.tile_pool(name="psU", bufs=3, space="PSUM"))
        psH = ctx.enter_context(tc.tile_pool(name="psH", bufs=2, space="PSUM"))
        psS = ctx.enter_context(tc.tile_pool(name="psS", bufs=1, space="PSUM"))
        psO = ctx.enter_context(tc.tile_pool(name="psO", bufs=1, space="PSUM"))

        # ---- constants / weights ----
        ident = cpool.tile([PC, PC], F32, tag="ident")
        make_identity(nc, ident[:])
        neghalf = cpool.tile([D, 1], F32, tag="neghalf")
        nc.vector.memset(neghalf[:], -0.5)

        xT = cpool.tile([D0, N], F32, tag="xT")
        nc.sync.dma_start(out=xT[:], in_=x_d[:].rearrange("n d -> d n"))

        def load_w(name, handle, shape):
            t = cpool.tile(list(shape), F32, tag=name)
            nc.sync.dma_start(out=t[:], in_=handle[:])
            return t

        W1s = load_w("W1s", W1_d, [D0, D])
        W2s = load_w("W2s", W2_d, [D, D])
        Wms = load_w("Wms", Wm_d, [D, D])
        # two base-partition-0 halves of Wu1 for the split-K (h | msgs) matmul
        Wu1a = cpool.tile([D, DU], F32, tag="Wu1a")
        nc.sync.dma_start(out=Wu1a[:], in_=Wu1_d[0:D, :])
        Wu1b = cpool.tile([D, DU], F32, tag="Wu1b")
        nc.sync.dma_start(out=Wu1b[:], in_=Wu1_d[D:DU, :])
        Wu2s = load_w("Wu2s", Wu2_d, [DU, DU])
        Wos = load_w("Wos", Wo_d, [DU, 3])

        def load_b(name, handle, p):
            t = cpool.tile([p, 1], F32, tag=name)
            nc.sync.dma_start(out=t[:], in_=handle[:, None])
            return t

        b1s = load_b("b1s", b1_d, D)
        b2s = load_b("b2s", b2_d, D)
        bms = load_b("bms", bm_d, D)
        bu1s = load_b("bu1s", bu1_d, DU)
        bu2s = load_b("bu2s", bu2_d, DU)
        bos = load_b("bos", bo_d, 3)

        # ---- persistent tiles ----
        # hoT rows 0..63 = h2T, row 64 = ones        (gram lhsT source)
        # hkT rows 0..63 = h2T, row 64 = -0.5*sq     (gram rhs source)
        hoT = pers.tile([D + 1, N], F32, tag="hoT")
        hkT = pers.tile([D + 1, N], F32, tag="hkT")
        h2rows = pers.tile([PC, NCH * D], F32, tag="h2rows")
        # [p, j, c]: j-th top-8 index for node c*128+p
        idx_all = pers.tile([PC, 8, NCH], U32, tag="idx_all")
        msum = pers.tile([D, N], F32, tag="msum")
        out3T = pers.tile([3, N], F32, tag="out3T")
        out_rows = pers.tile([PC, NCH * 3], F32, tag="out_rows")

        h_dram = dramp.tile([N, D], F32, tag="h_dram")

        nc.vector.memset(hoT[D : D + 1, :], 1.0)

        # ---- phase A: node MLP ----
        h1T = s64p.tile([D, N], F32, tag="s64")
        for q in range(QCH):
            qs = slice(q * NQ, (q + 1) * NQ)
            ps = psH.tile([D, NQ], F32, tag="psH")
            nc.tensor.matmul(out=ps[:], lhsT=W1s[:], rhs=xT[:, qs], start=True, stop=True)
            nc.scalar.activation(h1T[:, qs], ps[:], AF.Relu, bias=b1s[:])

        hsqT = s64p.tile([D, N], F32, tag="s64")
        for q in range(QCH):
            qs = slice(q * NQ, (q + 1) * NQ)
            ps = psH.tile([D, NQ], F32, tag="psH")
            nc.tensor.matmul(out=ps[:], lhsT=W2s[:], rhs=h1T[:, qs], start=True, stop=True)
            # two relu copies: one into hoT rows 0..63, one into hkT rows 0..63
            nc.scalar.activation(hoT[0:D, qs], ps[:], AF.Relu, bias=b2s[:])
            nc.scalar.activation(hkT[0:D, qs], ps[:], AF.Relu, bias=b2s[:])
            # hsq = h2^2
            nc.vector.tensor_mul(hsqT[:, qs], hoT[0:D, qs], hoT[0:D, qs])
            # -0.5 * sum_d hsq -> hkT row 64
            ps1 = psS.tile([1, NQ], F32, tag="psS")
            nc.tensor.matmul(out=ps1[:], lhsT=neghalf[:], rhs=hsqT[:, qs], start=True, stop=True)
            nc.scalar.copy(hkT[D : D + 1, qs], ps1[:])

        # h rows to DRAM for the neighbor gather: h2rows[p, c*64+d] = h[c*128+p, d]
        for half in range(2):
            pst = psU.tile([PC, NQ], F32, tag="psU")
            for i in range(8):
                c = half * 8 + i
                cs = slice(c * PC, (c + 1) * PC)
                nc.tensor.transpose(
                    out=pst[:, i * D : (i + 1) * D], in_=hoT[0:D, cs], identity=ident[0:D, 0:D]
                )
            nc.scalar.copy(h2rows[:, half * 8 * D : (half + 1) * 8 * D], pst[:])
        nc.sync.dma_start(
            out=h_dram[:].rearrange("(c p) d -> p c d", p=PC),
            in_=h2rows[:].rearrange("p (c d) -> p c d", c=NCH),
        )
        if dbg:
            nc.sync.dma_start(out=dbg_h[:], in_=h2rows[:])
            vals_all = pers.tile([PC, NCH, 8], F32, tag="vals_all")

        # ---- phase B: key matrix chunks + top-8 ----
        for c in range(NCH):
            cs = slice(c * PC, (c + 1) * PC)
            key_sb = keyp.tile([PC, N], F32, tag="key")
            for q in range(QCH):
                qs = slice(q * NQ, (q + 1) * NQ)
                ps = psU.tile([PC, NQ], F32, tag="psU")
                nc.tensor.matmul(out=ps[:], lhsT=hoT[:, cs], rhs=hkT[:, qs], start=True, stop=True)
                nc.scalar.copy(key_sb[:, qs], ps[:])
            vals8 = v8p.tile([PC, 8], F32, tag="v8")
            nc.vector.max(out=vals8[:], in_=key_sb[:])
            nc.vector.max_index(out=idx_all[:, :, c], in_max=vals8[:], in_values=key_sb[:])
            if dbg:
                nc.vector.tensor_copy(vals_all[:, c, :], vals8[:])
                if c == 0:
                    nc.sync.dma_start(out=dbg_key[:], in_=key_sb[:])
        if dbg:
            nc.sync.dma_start(out=dbg_vals[:], in_=vals_all[:])
            nc.sync.dma_start(out=dbg_idx[:], in_=idx_all[:])

        # ---- phase C: gather neighbors + message MLP (sum over k) ----
        # HW indirect DMA supports one offset per partition (vector mode), so
        # gather 128 neighbor rows per call: one call per (k, chunk).
        for k in range(KNN):
            neigh = neighp.tile([PC, NCH, D], F32, tag="neigh")
            for c in range(NCH):
                nc.gpsimd.indirect_dma_start(
                    out=neigh[:, c, :],
                    out_offset=None,
                    in_=h_dram[:],
                    in_offset=bass.IndirectOffsetOnAxis(
                        ap=idx_all[:, k + 1, c : c + 1], axis=0
                    ),
                )
            nT = ntp.tile([D, N], F32, tag="nT")
            for q in range(QCH):
                pst = psH.tile([D, NQ], F32, tag="psH")
                for i in range(4):
                    c = q * 4 + i
                    nc.tensor.transpose(
                        out=pst[:, i * PC : (i + 1) * PC], in_=neigh[:, c, :], identity=ident[:]
                    )
                nc.scalar.copy(nT[:, q * NQ : (q + 1) * NQ], pst[:])
            dst = msum if k == 0 else msgp.tile([D, N], F32, tag="msg")
            for q in range(QCH):
                qs = slice(q * NQ, (q + 1) * NQ)
                ps = psH.tile([D, NQ], F32, tag="psH")
                nc.tensor.matmul(out=ps[:], lhsT=Wms[:], rhs=nT[:, qs], start=True, stop=True)
                nc.scalar.activation(dst[:, qs], ps[:], AF.Relu, bias=bms[:])
            if dbg and k == 0:
                nc.sync.dma_start(out=dbg_neigh[:], in_=neigh[:])
            if k > 0:
                nc.vector.tensor_add(msum[:], msum[:], dst[:])
        if dbg:
            nc.sync.dma_start(out=dbg_msum[:], in_=msum[:])

        # ---- phase D: update MLP + output head ----
        u1T = up.tile([DU, N], F32, tag="u")
        for q in range(QCH):
            qs = slice(q * NQ, (q + 1) * NQ)
            ps = psU.tile([DU, NQ], F32, tag="psU")
            nc.tensor.matmul(out=ps[:], lhsT=Wu1a[:], rhs=hoT[0:D, qs], start=True, stop=False)
            nc.tensor.matmul(out=ps[:], lhsT=Wu1b[:], rhs=msum[:, qs], start=False, stop=True)
            nc.scalar.activation(u1T[:, qs], ps[:], AF.Relu, bias=bu1s[:])
        u2T = up.tile([DU, N], F32, tag="u")
        for q in range(QCH):
            qs = slice(q * NQ, (q + 1) * NQ)
            ps = psU.tile([DU, NQ], F32, tag="psU")
            nc.tensor.matmul(out=ps[:], lhsT=Wu2s[:], rhs=u1T[:, qs], start=True, stop=True)
            nc.scalar.activation(u2T[:, qs], ps[:], AF.Relu, bias=bu2s[:])
        for q in range(QCH):
            qs = slice(q * NQ, (q + 1) * NQ)
            ps = psS.tile([3, NQ], F32, tag="psS3")
            nc.tensor.matmul(out=ps[:], lhsT=Wos[:], rhs=u2T[:, qs], start=True, stop=True)
            nc.scalar.activation(out3T[:, qs], ps[:], AF.Identity, bias=bos[:])

        # ---- phase E: transpose out3T to rows and store ----
        pso = psO.tile([PC, NCH * 3], F32, tag="psO")
        for c in range(NCH):
            cs = slice(c * PC, (c + 1) * PC)
            nc.tensor.transpose(
                out=pso[:, c * 3 : (c + 1) * 3], in_=out3T[:, cs], identity=ident[0:3, 0:3]
            )
        nc.scalar.copy(out_rows[:], pso[:])
        nc.sync.dma_start(
            out=out3_d[:].rearrange("(c p) e -> p c e", p=PC),
            in_=out_rows[:].rearrange("p (c e) -> p c e", c=NCH),
        )

    nc.compile()
    return nc


_CACHED = None


def _get_program():
    global _CACHED
    if _CACHED is None:
        _CACHED = build_program()
    return _CACHED


def make_in_maps(inputs):
    f = lambda a: np.ascontiguousarray(np.asarray(a, dtype=np.float32))
    x = f(inputs["x"])
    Wu1 = f(inputs["Wu1"]).copy()
    Wu1[D:DU, :] /= 3.0  # fold mean over 3 neighbors into the update weights
    Wo = np.concatenate([f(inputs["Wmean"]), f(inputs["Wv"])], axis=1)
    bo = np.concatenate([f(inputs["bmean"]), f(inputs["bv"])], axis=0)
    shared = {
        "W1": f(inputs["W1"]), "b1": f(inputs["b1"]),
        "W2": f(inputs["W2"]), "b2": f(inputs["b2"]),
        "Wm": f(inputs["Wm"]), "bm": f(inputs["bm"]),
        "Wu1": Wu1, "bu1": f(inputs["bu1"]),
        "Wu2": f(inputs["Wu2"]), "bu2": f(inputs["bu2"]),
        "Wo": Wo, "bo": bo,
    }
    return [dict(shared, x=x[b]) for b in range(B)]


def kernel(**inputs):
    from concourse.bass_utils import run_bass_kernel_spmd

    nc = _get_program()
    in_maps = make_in_maps(inputs)
    res = run_bass_kernel_spmd(nc, in_maps, core_ids=list(range(B)))
    out3 = np.stack([res.results[b]["out3"] for b in range(B)], axis=0)  # [B, N, 3]
    mean = np.ascontiguousarray(out3[:, :, 0:2])
    v = np.ascontiguousarray(out3[:, :, 2:3])
    std = np.exp(np.asarray(inputs["log_std"], dtype=np.float32))
    return mean, std, v
